# revision 15
# baseline (speedup 1.0000x reference)
"""Trainium2 Bass kernel for nn_DecoderRNN (show-attend-tell style decoder).

Math restructuring exploited here:
  - The attention logit h-term (h @ Wa.T + ba) is constant over the 196
    spatial locations, so it cancels in softmax(axis=locations).  Hence
    alpha and ctx are the SAME for every timestep -> computed once.
  - Therefore gates_t = [ctx@W_ihc.T + emb_t@W_ihe.T + b_ih + b_hh]  (static,
    precomputed for all t) + h_t @ W_hh.T  (the only per-step matmul).
  - bv and ba cancel in their softmaxes and are dropped.

Precision: matmul operands are bf16 (PE single-pass + fast weight load),
accumulation and pointwise math fp32.  Attention logits att_v are computed
from fp32 features (softmax weights are error-amplifying); the weighted
values G = F*alpha are bf16 (roundings average out over 196 locations).

Sharding: data-parallel over batch (128 -> 16 per core x 8 cores).
Gate order is host-permuted to (g, i, f, o) so tanh/sigmoid splits are
contiguous psum slices that pipeline under the matmuls.
"""

import functools
import os
import sys

import numpy as np

if "/opt/trn_rl_repo" not in sys.path:
    sys.path.insert(0, "/opt/trn_rl_repo")

_PHASES = int(os.environ.get("KPHASES", "2"))  # debug: 0/1/2 = stop after phase

# Problem constants (hardcoded per contract)
B, T = 128, 20
NCORES, BSH = 8, 16  # batch shard per core
NVIS, NHI, NLO = 196, 8, 25  # 196 locations padded to 8*25=200
VD, ED, H, G4, VOC = 512, 256, 512, 2048, 10000
VT, NVT = 500, 20  # vocab tile size for phase 2
ROWS = T * BSH  # 320 output rows per core
CHUNKS = [(0, 128), (128, 128), (256, 64)]  # phase-2 row chunks


@functools.lru_cache(maxsize=1)
def _build_nc():
    import concourse.bass as bass
    import concourse.tile as tile
    from concourse import bacc, mybir
    from contextlib import ExitStack

    FP = mybir.dt.float32
    BF = mybir.dt.bfloat16
    AF = mybir.ActivationFunctionType
    OP = mybir.AluOpType
    AX = mybir.AxisListType

    nc = bacc.Bacc("TRN2", target_bir_lowering=False, debug=False, num_devices=NCORES)

    d_f = nc.dram_tensor("f", [128, NLO, VD], FP, kind="ExternalInput").ap()
    d_embt = nc.dram_tensor("embt", [128, 2, T, BSH], BF, kind="ExternalInput").ap()
    d_whh = nc.dram_tensor("whh", [128, 4, G4], BF, kind="ExternalInput").ap()
    d_wihe = nc.dram_tensor("wihe", [128, 2, G4], BF, kind="ExternalInput").ap()
    d_wihc = nc.dram_tensor("wihc", [128, 4, G4], BF, kind="ExternalInput").ap()
    d_winh = nc.dram_tensor("winh", [128, 4, H], BF, kind="ExternalInput").ap()
    d_winc = nc.dram_tensor("winc", [128, 4, H], BF, kind="ExternalInput").ap()
    d_wot = nc.dram_tensor("wot", [128, 4, VOC], BF, kind="ExternalInput").ap()
    d_biasrow = nc.dram_tensor("biasrow", [1, G4], BF, kind="ExternalInput").ap()
    d_borow = nc.dram_tensor("borow", [1, VOC], BF, kind="ExternalInput").ap()
    d_wvb = nc.dram_tensor("wvb", [128, VD], FP, kind="ExternalInput").ap()
    d_onesbd = nc.dram_tensor("onesbd", [128, BSH], BF, kind="ExternalInput").ap()
    d_i16 = nc.dram_tensor("i16", [BSH, BSH], BF, kind="ExternalInput").ap()
    d_onesrow = nc.dram_tensor("onesrow", [1, 128], BF, kind="ExternalInput").ap()
    d_padmask = nc.dram_tensor("padmask", [128, NLO], FP, kind="ExternalInput").ap()
    d_lsm = nc.dram_tensor("out_lsm", [ROWS, VOC], FP, kind="ExternalOutput").ap()
    d_sm = nc.dram_tensor("out_sm", [ROWS, VOC], FP, kind="ExternalOutput").ap()
    d_ge = nc.dram_tensor("ge_scratch", [ROWS, G4], BF, kind="Internal").ap()

    with tile.TileContext(nc) as tc, ExitStack() as whole:
        singles = whole.enter_context(tc.tile_pool(name="singles", bufs=1))
        sb_onesbd = singles.tile([128, BSH], BF)
        nc.sync.dma_start(out=sb_onesbd, in_=d_onesbd)
        sb_i16 = singles.tile([BSH, BSH], BF)
        nc.sync.dma_start(out=sb_i16, in_=d_i16)
        sb_onesrow = singles.tile([1, 128], BF)
        nc.sync.dma_start(out=sb_onesrow, in_=d_onesrow)
        # transposed h history (bf16): slot 0 = h0, slot t+1 = h after step t
        hallT = singles.tile([128, 4, BSH * (T + 1)], BF)
        c_sb = singles.tile([BSH, H], FP)
        h_sb = singles.tile([BSH, H], BF)

        with ExitStack() as p01:
            wpool = p01.enter_context(tc.tile_pool(name="wpool", bufs=1))
            sb_embt = wpool.tile([128, 2, T, BSH], BF)
            nc.sync.dma_start(out=sb_embt, in_=d_embt)

            # ---------------- phase 0: static attention + GE precompute ----
            with ExitStack() as p0:
                f0 = p0.enter_context(tc.tile_pool(name="f0", bufs=1))
                w0 = p0.enter_context(tc.tile_pool(name="w0", bufs=1))
                g0 = p0.enter_context(tc.tile_pool(name="g0", bufs=3))
                ps0 = p0.enter_context(tc.tile_pool(name="ps0", bufs=1, space="PSUM"))
                tps0 = p0.enter_context(tc.tile_pool(name="tps0", bufs=2, space="PSUM"))

                f_sb = f0.tile([128, NLO, VD], FP)
                for j in range(5):
                    nc.sync.dma_start(
                        out=f_sb[:, j * 5 : (j + 1) * 5, :],
                        in_=d_f[:, j * 5 : (j + 1) * 5, :],
                    )
                sb_wvb = w0.tile([128, VD], FP)
                nc.sync.dma_start(out=sb_wvb, in_=d_wvb)
                sb_padmask = w0.tile([128, NLO], FP)
                nc.sync.dma_start(out=sb_padmask, in_=d_padmask)
                sb_biasrow = w0.tile([1, G4], BF)
                nc.sync.dma_start(out=sb_biasrow, in_=d_biasrow)
                sb_wihe = w0.tile([128, 2, G4], BF)
                nc.sync.dma_start(out=sb_wihe, in_=d_wihe)
                sb_wihc = w0.tile([128, 4, G4], BF)
                nc.sync.dma_start(out=sb_wihc, in_=d_wihc)
                sb_winh = w0.tile([128, 4, H], BF)
                nc.sync.dma_start(out=sb_winh, in_=d_winh)
                sb_winc = w0.tile([128, 4, H], BF)
                nc.sync.dma_start(out=sb_winc, in_=d_winc)

                # attention logits att_v = F . Wv  (fp32; per (b, n) row)
                attv = w0.tile([128, NLO], FP)
                for nlo in range(NLO):
                    gsc = g0.tile([128, VD], FP, name="gf")
                    nc.vector.tensor_mul(out=gsc, in0=f_sb[:, nlo, :], in1=sb_wvb)
                    nc.vector.tensor_reduce(
                        out=attv[:, nlo : nlo + 1], in_=gsc, axis=AX.X, op=OP.add
                    )
                # E = exp(att_v) * padmask   (max-sub skipped: |att_v| < ~3)
                e_sb = w0.tile([128, NLO], FP)
                nc.scalar.activation(out=e_sb, in_=attv, func=AF.Exp)
                nc.vector.tensor_mul(out=e_sb, in0=e_sb, in1=sb_padmask)
                esum = w0.tile([128, 1], FP)
                nc.vector.tensor_reduce(out=esum, in_=e_sb, axis=AX.X, op=OP.add)
                esum_bf = w0.tile([128, 1], BF)
                nc.vector.tensor_copy(out=esum_bf, in_=esum)
                den_ps = ps0.tile([BSH, 1], FP, tag="ps_a")
                nc.tensor.matmul(
                    den_ps, lhsT=sb_onesbd, rhs=esum_bf, start=True, stop=True
                )
                rden = w0.tile([BSH, 1], FP)
                nc.vector.reciprocal(out=rden, in_=den_ps)

                # fbar-sum via one big DVE reduce + one block-diag matmul
                fsum = w0.tile([128, VD], FP)
                f_v_nlo = bass.AP(
                    tensor=f_sb.tensor,
                    offset=f_sb.offset,
                    ap=[f_sb.ap[0], f_sb.ap[2], f_sb.ap[1]],  # [p, v, nlo]
                )
                nc.vector.tensor_reduce(out=fsum, in_=f_v_nlo, axis=AX.X, op=OP.add)
                fsum_bf = w0.tile([128, VD], BF)
                nc.vector.tensor_copy(out=fsum_bf, in_=fsum)
                fb_ps = ps0.tile([BSH, VD], FP, tag="ps_b")
                nc.tensor.matmul(fb_ps, lhsT=sb_onesbd, rhs=fsum_bf, start=True, stop=True)

                # ctx (unnormalized): G = F*E (bf16), block-diag-ones matmul
                ctx_ps = ps0.tile([BSH, VD], FP, tag="ps_a")
                for nlo in range(NLO):
                    g = g0.tile([128, VD], BF, name="g")
                    nc.vector.tensor_scalar_mul(
                        out=g, in0=f_sb[:, nlo, :], scalar1=e_sb[:, nlo : nlo + 1]
                    )
                    nc.tensor.matmul(
                        ctx_ps, lhsT=sb_onesbd, rhs=g,
                        start=(nlo == 0), stop=(nlo == NLO - 1),
                    )
                ctx_sb = w0.tile([BSH, VD], BF)
                nc.vector.tensor_scalar_mul(out=ctx_sb, in0=ctx_ps, scalar1=rden)
                fb_sb = w0.tile([BSH, VD], BF)
                nc.scalar.activation(
                    out=fb_sb, in_=fb_ps, func=AF.Copy, scale=1.0 / float(NVIS)
                )

                # transpose ctx and fbar -> [512(4x128), 16] via PE transpose
                ctxT = w0.tile([128, 4, BSH], BF)
                fbT = w0.tile([128, 4, BSH], BF)
                for src, dst in ((ctx_sb, ctxT), (fb_sb, fbT)):
                    tp = tps0.tile([128, 4 * BSH], BF, name="tp")
                    for kt in range(4):
                        nc.tensor.transpose(
                            tp[:, kt * BSH : (kt + 1) * BSH],
                            src[:, kt * 128 : (kt + 1) * 128],
                            sb_i16,
                        )
                    nc.scalar.copy(out=dst, in_=tp.rearrange("p (k b) -> p k b", k=4))

                # h0 / c0 = (fbar) @ W_init^T
                h0_ps = ps0.tile([BSH, H], FP, tag="ps_a")
                c0_ps = ps0.tile([BSH, H], FP, tag="ps_b")
                for kt in range(4):
                    nc.tensor.matmul(
                        h0_ps, lhsT=fbT[:, kt, :], rhs=sb_winh[:, kt, :],
                        start=(kt == 0), stop=(kt == 3),
                    )
                for kt in range(4):
                    nc.tensor.matmul(
                        c0_ps, lhsT=fbT[:, kt, :], rhs=sb_winc[:, kt, :],
                        start=(kt == 0), stop=(kt == 3),
                    )
                nc.scalar.copy(out=c_sb, in_=c0_ps)
                h0_sb = w0.tile([BSH, H], BF)
                nc.scalar.copy(out=h0_sb, in_=h0_ps)
                tp0 = tps0.tile([128, 4 * BSH], BF, name="tp")
                for kt in range(4):
                    nc.tensor.transpose(
                        tp0[:, kt * BSH : (kt + 1) * BSH],
                        h0_sb[:, kt * 128 : (kt + 1) * 128],
                        sb_i16,
                    )
                nc.scalar.copy(
                    out=hallT[:, :, 0:BSH],
                    in_=tp0.rearrange("p (k b) -> p k b", k=4),
                )

                # ctx columns replicated across t: [128, 4, T*BSH]
                ctxRepT = w0.tile([128, 4, T * BSH], BF)
                ctx_b = bass.AP(
                    tensor=ctxT.tensor,
                    offset=ctxT.offset,
                    ap=[ctxT.ap[0], ctxT.ap[1], [0, T], ctxT.ap[2]],
                )
                nc.vector.tensor_copy(
                    out=ctxRepT.rearrange("p k (t b) -> p k t b", t=T), in_=ctx_b
                )
                embt_flat = sb_embt.rearrange("p a t b -> p (a t b)")

                # GE[t*16+b, :] = ctx@W_ihc^T + emb_t@W_ihe^T + (b_ih + b_hh)
                for m0, ml in CHUNKS:
                    ge_ps = ps0.tile([128, G4], FP, tag="ps_big")
                    for ns in range(4):
                        nsl = slice(ns * 512, (ns + 1) * 512)
                        for et in range(2):
                            e0 = et * T * BSH + m0
                            nc.tensor.matmul(
                                ge_ps[0:ml, nsl],
                                lhsT=embt_flat[:, e0 : e0 + ml],
                                rhs=sb_wihe[:, et, nsl],
                                start=(et == 0), stop=False,
                            )
                        for kt in range(4):
                            nc.tensor.matmul(
                                ge_ps[0:ml, nsl],
                                lhsT=ctxRepT[:, kt, m0 : m0 + ml],
                                rhs=sb_wihc[:, kt, nsl],
                                start=False, stop=False,
                            )
                        nc.tensor.matmul(
                            ge_ps[0:ml, nsl],
                            lhsT=sb_onesrow[0:1, 0:ml],
                            rhs=sb_biasrow[0:1, nsl],
                            start=False, stop=True,
                        )
                    ge_sb = g0.tile([128, G4], BF, name="ge_sb", bufs=2)
                    nc.scalar.copy(out=ge_sb[0:ml, :], in_=ge_ps[0:ml, :])
                    nc.sync.dma_start(out=d_ge[m0 : m0 + ml, :], in_=ge_sb[0:ml, :])

            # ---------------- phase 1: LSTM recurrence -----------------
            # gate order after host permutation: (g, i, f, o)
            with ExitStack() as p1:
                whp = p1.enter_context(tc.tile_pool(name="whp", bufs=1))
                sb_whh = whp.tile([128, 4, G4], BF)
                nc.sync.dma_start(out=sb_whh, in_=d_whh)
                gein = p1.enter_context(tc.tile_pool(name="gein", bufs=3))
                gps = p1.enter_context(tc.tile_pool(name="gps", bufs=1, space="PSUM"))
                tps1 = p1.enter_context(tc.tile_pool(name="tps1", bufs=2, space="PSUM"))
                wps = p1.enter_context(tc.tile_pool(name="wps", bufs=1, space="PSUM"))
                apool = p1.enter_context(tc.tile_pool(name="apool", bufs=2))

                warm_ps = wps.tile([BSH, BSH], FP)

                for t in range(T if _PHASES >= 1 else 0):
                    ge_t = gein.tile([BSH, G4], BF, name="ge_t")
                    nc.sync.dma_start(out=ge_t, in_=d_ge[t * BSH : (t + 1) * BSH, :])
                    gates = gps.tile([BSH, G4], FP, name="gates")
                    hsl = slice(t * BSH, (t + 1) * BSH)
                    acts = {}
                    ig = apool.tile([BSH, H], FP, name="ig")
                    for ns in range(4):
                        nsl = slice(ns * 512, (ns + 1) * 512)
                        for kt in range(4):
                            nc.tensor.matmul(
                                gates[:, nsl],
                                lhsT=hallT[:, kt, hsl],
                                rhs=sb_whh[:, kt, nsl],
                                start=(kt == 0), stop=False,
                            )
                        nc.tensor.matmul(
                            gates[:, nsl], lhsT=sb_i16, rhs=ge_t[:, nsl],
                            start=False, stop=True,
                        )
                        # pointwise for this gate slice, pipelined under
                        # the next slice's matmuls
                        gt = apool.tile([BSH, H], FP, name=f"act{ns}")
                        fn = AF.Tanh if ns == 0 else AF.Sigmoid
                        nc.scalar.activation(out=gt, in_=gates[:, nsl], func=fn)
                        acts[ns] = gt
                        if ns == 1:
                            nc.vector.tensor_mul(out=ig, in0=acts[1], in1=acts[0])
                        elif ns == 2:
                            nc.vector.tensor_mul(out=c_sb, in0=acts[2], in1=c_sb)
                    # tiny matmul chained off ig keeps the PE HAM clock warm
                    nc.tensor.matmul(
                        warm_ps, lhsT=ig[0:BSH, 0:BSH], rhs=ig[0:BSH, 0:BSH],
                        start=True, stop=True,
                    )
                    nc.vector.tensor_add(out=c_sb, in0=c_sb, in1=ig)
                    th = apool.tile([BSH, H], FP, name="th")
                    nc.scalar.activation(out=th, in_=c_sb, func=AF.Tanh)
                    nc.vector.tensor_mul(out=h_sb, in0=acts[3], in1=th)
                    nc.tensor.matmul(
                        warm_ps, lhsT=th[0:BSH, 0:BSH], rhs=th[0:BSH, 0:BSH],
                        start=True, stop=True,
                    )
                    tp1 = tps1.tile([128, 4 * BSH], BF, name="tp1")
                    for kt in range(4):
                        nc.tensor.transpose(
                            tp1[:, kt * BSH : (kt + 1) * BSH],
                            h_sb[:, kt * 128 : (kt + 1) * 128],
                            sb_i16,
                        )
                    nc.scalar.copy(
                        out=hallT[:, :, (t + 1) * BSH : (t + 2) * BSH],
                        in_=tp1.rearrange("p (k b) -> p k b", k=4),
                    )

        # ---------------- phase 2: vocab projection + softmax ----------
        with ExitStack() as p2:
            ep = p2.enter_context(tc.tile_pool(name="ep", bufs=1))
            wop = p2.enter_context(tc.tile_pool(name="wop", bufs=4))
            bop = p2.enter_context(tc.tile_pool(name="bop", bufs=4))
            ps2 = p2.enter_context(tc.tile_pool(name="ps2", bufs=6, space="PSUM"))
            sp = p2.enter_context(tc.tile_pool(name="sp", bufs=1))

            ebs, scols = [], []
            for ci in range(3):
                eb = ep.tile([128, VOC], FP, name=f"eb{ci}")
                sc = sp.tile([128, NVT], FP, name=f"sc{ci}")
                ebs.append(eb)
                scols.append(sc)

            for vt in range(NVT if _PHASES >= 2 else 0):
                vsl = slice(vt * VT, (vt + 1) * VT)
                wo_t = wop.tile([128, 4, VT], BF, name="wo_t")
                nc.sync.dma_start(out=wo_t, in_=d_wot[:, :, vsl])
                bo_t = bop.tile([1, VT], BF, name="bo_t")
                nc.sync.dma_start(out=bo_t, in_=d_borow[0:1, vsl])
                for ci, (m0, ml) in enumerate(CHUNKS):
                    ps = ps2.tile([128, VT], FP, name="ps")
                    for kt in range(4):
                        nc.tensor.matmul(
                            ps[0:ml, :],
                            lhsT=hallT[:, kt, BSH + m0 : BSH + m0 + ml],
                            rhs=wo_t[:, kt, :],
                            start=(kt == 0), stop=False,
                        )
                    nc.tensor.matmul(
                        ps[0:ml, :], lhsT=sb_onesrow[0:1, 0:ml], rhs=bo_t,
                        start=False, stop=True,
                    )
                    nc.scalar.activation(
                        out=ebs[ci][0:ml, vsl],
                        in_=ps[0:ml, :],
                        func=AF.Exp,
                        accum_out=scols[ci][0:ml, vt : vt + 1],
                    )
            for ci, (m0, ml) in enumerate(CHUNKS if _PHASES >= 2 else []):
                s_t = sp.tile([128, 1], FP, name=f"s{ci}")
                nc.vector.tensor_reduce(
                    out=s_t[0:ml], in_=scols[ci][0:ml, :], axis=AX.X, op=OP.add
                )
                r_t = sp.tile([128, 1], FP, name=f"r{ci}")
                nc.vector.reciprocal(out=r_t[0:ml], in_=s_t[0:ml])
                nc.vector.tensor_scalar_mul(
                    out=ebs[ci][0:ml, :], in0=ebs[ci][0:ml, :], scalar1=r_t[0:ml]
                )
                for hf in range(2):
                    fsl = slice(hf * 5000, (hf + 1) * 5000)
                    nc.sync.dma_start(
                        out=d_sm[m0 : m0 + ml, fsl], in_=ebs[ci][0:ml, fsl]
                    )
                nc.scalar.activation(
                    out=ebs[ci][0:ml, :], in_=ebs[ci][0:ml, :], func=AF.Ln
                )
                for hf in range(2):
                    fsl = slice(hf * 5000, (hf + 1) * 5000)
                    nc.sync.dma_start(
                        out=d_lsm[m0 : m0 + ml, fsl], in_=ebs[ci][0:ml, fsl]
                    )

    nc.compile()
    return nc


def _prep_host(inputs):
    import ml_dtypes

    f32 = np.float32
    bf16 = ml_dtypes.bfloat16
    feats = np.asarray(inputs["features"], f32)  # [128,196,512]
    caps = np.asarray(inputs["captions"]).astype(np.int64)
    emb_table = np.asarray(inputs["embed_table"], f32)
    emb = emb_table[caps]  # [128,20,256]

    W_ih = np.asarray(inputs["W_ih"], f32)  # [2048, 768]
    W_hh = np.asarray(inputs["W_hh"], f32)  # [2048, 512]
    Wo = np.asarray(inputs["Wo"], f32)  # [10000, 512]

    # permute gate rows: torch (i, f, g, o) -> (g, i, f, o)
    perm = np.concatenate(
        [np.arange(1024, 1536), np.arange(0, 512), np.arange(512, 1024),
         np.arange(1536, 2048)]
    )
    W_ih = W_ih[perm]
    W_hh = W_hh[perm]
    bias = (np.asarray(inputs["b_ih"], f32) + np.asarray(inputs["b_hh"], f32))[perm]

    def kxm(w_t, ktiles, ncols, dt=bf16):
        # w_t: [K, N] (already transposed weight) -> [128, ktiles, N]
        return np.ascontiguousarray(
            w_t.reshape(ktiles, 128, ncols).transpose(1, 0, 2).astype(dt)
        )

    shared = {
        "whh": kxm(W_hh.T.copy(), 4, G4),
        "wihe": kxm(np.ascontiguousarray(W_ih[:, VD:].T), 2, G4),
        "wihc": kxm(np.ascontiguousarray(W_ih[:, :VD].T), 4, G4),
        "winh": kxm(np.asarray(inputs["W_init_h"], f32).T.copy(), 4, H),
        "winc": kxm(np.asarray(inputs["W_init_c"], f32).T.copy(), 4, H),
        "wot": kxm(Wo.T.copy(), 4, VOC),
        "biasrow": np.ascontiguousarray(bias.reshape(1, G4).astype(bf16)),
        "borow": np.ascontiguousarray(
            np.asarray(inputs["bo"], f32).reshape(1, VOC).astype(bf16)
        ),
        "wvb": np.ascontiguousarray(
            np.broadcast_to(np.asarray(inputs["Wv"], f32).reshape(1, VD), (128, VD))
        ),
        "onesbd": np.ascontiguousarray(
            (np.arange(128)[:, None] // NHI == np.arange(BSH)[None, :]).astype(bf16)
        ),
        "i16": np.eye(BSH, dtype=bf16),
        "onesrow": np.ones((1, 128), bf16),
        "padmask": np.ascontiguousarray(
            (
                (np.arange(128)[:, None] % NHI) * NLO + np.arange(NLO)[None, :] < NVIS
            ).astype(f32)
        ),
    }

    in_maps = []
    for c in range(NCORES):
        fc = feats[c * BSH : (c + 1) * BSH]  # [16,196,512]
        fpad = np.zeros((BSH, NHI * NLO, VD), f32)
        fpad[:, :NVIS] = fc
        f_host = np.ascontiguousarray(fpad.reshape(128, NLO, VD))
        emb_c = emb[c * BSH : (c + 1) * BSH]  # [16,20,256]
        embt = np.ascontiguousarray(
            emb_c.transpose(2, 1, 0)
            .reshape(2, 128, T, BSH)
            .transpose(1, 0, 2, 3)
            .astype(bf16)
        )
        in_maps.append({"f": f_host, "embt": embt, **shared})
    return in_maps


def run_with_results(inputs, trace=False):
    from concourse.bass_utils import run_bass_kernel_spmd

    nc = _build_nc()
    in_maps = _prep_host(inputs)
    res = run_bass_kernel_spmd(
        nc, in_maps, core_ids=list(range(NCORES)), trace=trace
    )
    lsm_cores = np.stack([r["out_lsm"] for r in res.results])  # [8, 320, 10000]
    sm_cores = np.stack([r["out_sm"] for r in res.results])

    def assemble(a):
        # [8 cores, 20*16, V] -> time-major [T*B, V] with row = t*128 + b_global
        return np.ascontiguousarray(
            a.reshape(NCORES, T, BSH, VOC).transpose(1, 0, 2, 3).reshape(T * B, VOC)
        )

    return (assemble(lsm_cores), assemble(sm_cores)), res


def kernel(**inputs):
    outs, _ = run_with_results(inputs, trace=False)
    return outs


# revision 17
# speedup vs baseline: 1.0109x; 1.0109x over previous
"""Trainium2 Bass kernel for nn_DecoderRNN (show-attend-tell style decoder).

Math restructuring exploited here:
  - The attention logit h-term (h @ Wa.T + ba) is constant over the 196
    spatial locations, so it cancels in softmax(axis=locations).  Hence
    alpha and ctx are the SAME for every timestep -> computed once.
  - Therefore gates_t = [ctx@W_ihc.T + emb_t@W_ihe.T + b_ih + b_hh]  (static,
    precomputed for all t) + h_t @ W_hh.T  (the only per-step matmul).
  - bv and ba cancel in their softmaxes and are dropped.

Precision: matmul operands are bf16 (PE single-pass + fast weight load),
accumulation and pointwise math fp32.  Attention logits att_v are computed
from fp32 features (softmax weights are error-amplifying); the weighted
values G = F*alpha are bf16 (roundings average out over 196 locations).

Scheduling: the vocab projection (phase 2) is interleaved into the LSTM
steps as soon as each 128-row output chunk's h states exist — this keeps
TensorE dense (HAM stays at full clock) and hides the output-side DMA.
Wo is streamed once per row-chunk (3x, bf16).

Sharding: data-parallel over batch (128 -> 16 per core x 8 cores).
Gate order is host-permuted to (g, i, f, o) so tanh/sigmoid splits are
contiguous psum slices that pipeline under the matmuls.
"""

import functools
import sys

import numpy as np

if "/opt/trn_rl_repo" not in sys.path:
    sys.path.insert(0, "/opt/trn_rl_repo")

# Problem constants (hardcoded per contract)
B, T = 128, 20
NCORES, BSH = 8, 16  # batch shard per core
NVIS, NHI, NLO = 196, 8, 25  # 196 locations padded to 8*25=200
VD, ED, H, G4, VOC = 512, 256, 512, 2048, 10000
VT, NVT = 500, 20  # vocab tile size for phase 2
ROWS = T * BSH  # 320 output rows per core
CHUNKS = [(0, 128), (128, 128), (256, 64)]  # phase-2 row chunks


@functools.lru_cache(maxsize=1)
def _build_nc():
    import concourse.bass as bass
    import concourse.tile as tile
    from concourse import bacc, mybir
    from contextlib import ExitStack

    FP = mybir.dt.float32
    BF = mybir.dt.bfloat16
    AF = mybir.ActivationFunctionType
    OP = mybir.AluOpType
    AX = mybir.AxisListType

    nc = bacc.Bacc("TRN2", target_bir_lowering=False, debug=False, num_devices=NCORES)

    d_f = nc.dram_tensor("f", [128, NLO, VD], FP, kind="ExternalInput").ap()
    d_embt = nc.dram_tensor("embt", [128, 2, T, BSH], BF, kind="ExternalInput").ap()
    d_whh = nc.dram_tensor("whh", [128, 4, G4], BF, kind="ExternalInput").ap()
    d_wihe = nc.dram_tensor("wihe", [128, 2, G4], BF, kind="ExternalInput").ap()
    d_wihc = nc.dram_tensor("wihc", [128, 4, G4], BF, kind="ExternalInput").ap()
    d_winh = nc.dram_tensor("winh", [128, 4, H], BF, kind="ExternalInput").ap()
    d_winc = nc.dram_tensor("winc", [128, 4, H], BF, kind="ExternalInput").ap()
    d_wot = nc.dram_tensor("wot", [128, 4, VOC], BF, kind="ExternalInput").ap()
    d_biasrow = nc.dram_tensor("biasrow", [1, G4], BF, kind="ExternalInput").ap()
    d_borow = nc.dram_tensor("borow", [1, VOC], BF, kind="ExternalInput").ap()
    d_wvb = nc.dram_tensor("wvb", [128, VD], FP, kind="ExternalInput").ap()
    d_onesbd = nc.dram_tensor("onesbd", [128, BSH], BF, kind="ExternalInput").ap()
    d_i16 = nc.dram_tensor("i16", [BSH, BSH], BF, kind="ExternalInput").ap()
    d_onesrow = nc.dram_tensor("onesrow", [1, 128], BF, kind="ExternalInput").ap()
    d_padmask = nc.dram_tensor("padmask", [128, NLO], FP, kind="ExternalInput").ap()
    d_lsm = nc.dram_tensor("out_lsm", [ROWS, VOC], FP, kind="ExternalOutput").ap()
    d_sm = nc.dram_tensor("out_sm", [ROWS, VOC], FP, kind="ExternalOutput").ap()
    d_ge = nc.dram_tensor("ge_scratch", [ROWS, G4], BF, kind="Internal").ap()

    with tile.TileContext(nc) as tc, ExitStack() as whole:
        singles = whole.enter_context(tc.tile_pool(name="singles", bufs=1))
        sb_onesbd = singles.tile([128, BSH], BF)
        nc.sync.dma_start(out=sb_onesbd, in_=d_onesbd)
        sb_i16 = singles.tile([BSH, BSH], BF)
        nc.sync.dma_start(out=sb_i16, in_=d_i16)
        sb_onesrow = singles.tile([1, 128], BF)
        nc.sync.dma_start(out=sb_onesrow, in_=d_onesrow)
        # transposed h history (bf16): slot 0 = h0, slot t+1 = h after step t
        hallT = singles.tile([128, 4, BSH * (T + 1)], BF)
        c_sb = singles.tile([BSH, H], FP)
        h_sb = singles.tile([BSH, H], BF)

        # ---------------- phase 0: static attention + GE precompute ----
        with ExitStack() as p0:
            f0 = p0.enter_context(tc.tile_pool(name="f0", bufs=1))
            w0 = p0.enter_context(tc.tile_pool(name="w0", bufs=1))
            g0 = p0.enter_context(tc.tile_pool(name="g0", bufs=3))
            ps0 = p0.enter_context(tc.tile_pool(name="ps0", bufs=1, space="PSUM"))
            tps0 = p0.enter_context(tc.tile_pool(name="tps0", bufs=2, space="PSUM"))

            f_sb = f0.tile([128, NLO, VD], FP)
            for j in range(5):
                nc.sync.dma_start(
                    out=f_sb[:, j * 5 : (j + 1) * 5, :],
                    in_=d_f[:, j * 5 : (j + 1) * 5, :],
                )
            sb_wvb = w0.tile([128, VD], FP)
            nc.sync.dma_start(out=sb_wvb, in_=d_wvb)
            sb_padmask = w0.tile([128, NLO], FP)
            nc.sync.dma_start(out=sb_padmask, in_=d_padmask)
            sb_biasrow = w0.tile([1, G4], BF)
            nc.sync.dma_start(out=sb_biasrow, in_=d_biasrow)
            sb_wihe = w0.tile([128, 2, G4], BF)
            nc.sync.dma_start(out=sb_wihe, in_=d_wihe)
            sb_wihc = w0.tile([128, 4, G4], BF)
            nc.sync.dma_start(out=sb_wihc, in_=d_wihc)
            sb_winh = w0.tile([128, 4, H], BF)
            nc.sync.dma_start(out=sb_winh, in_=d_winh)
            sb_winc = w0.tile([128, 4, H], BF)
            nc.sync.dma_start(out=sb_winc, in_=d_winc)
            sb_embt = w0.tile([128, 2, T, BSH], BF)
            nc.sync.dma_start(out=sb_embt, in_=d_embt)

            # attention logits att_v = F . Wv  (fp32; per (b, n) row)
            attv = w0.tile([128, NLO], FP)
            for nlo in range(NLO):
                gsc = g0.tile([128, VD], FP, name="gf")
                nc.vector.tensor_mul(out=gsc, in0=f_sb[:, nlo, :], in1=sb_wvb)
                nc.vector.tensor_reduce(
                    out=attv[:, nlo : nlo + 1], in_=gsc, axis=AX.X, op=OP.add
                )

            # meanwhile on PE/ACT: fbar -> h0/c0 (independent of attention)
            fsum = w0.tile([128, VD], FP)
            f_v_nlo = bass.AP(
                tensor=f_sb.tensor,
                offset=f_sb.offset,
                ap=[f_sb.ap[0], f_sb.ap[2], f_sb.ap[1]],  # [p, v, nlo]
            )
            nc.vector.tensor_reduce(out=fsum, in_=f_v_nlo, axis=AX.X, op=OP.add)
            fsum_bf = w0.tile([128, VD], BF)
            nc.vector.tensor_copy(out=fsum_bf, in_=fsum)
            fb_ps = ps0.tile([BSH, VD], FP, tag="ps_b")
            nc.tensor.matmul(fb_ps, lhsT=sb_onesbd, rhs=fsum_bf, start=True, stop=True)
            fb_sb = w0.tile([BSH, VD], BF)
            nc.scalar.activation(
                out=fb_sb, in_=fb_ps, func=AF.Copy, scale=1.0 / float(NVIS)
            )
            fbT = w0.tile([128, 4, BSH], BF)
            tpf = tps0.tile([128, 4 * BSH], BF, name="tp")
            for kt in range(4):
                nc.tensor.transpose(
                    tpf[:, kt * BSH : (kt + 1) * BSH],
                    fb_sb[:, kt * 128 : (kt + 1) * 128],
                    sb_i16,
                )
            nc.scalar.copy(out=fbT, in_=tpf.rearrange("p (k b) -> p k b", k=4))
            h0_ps = ps0.tile([BSH, H], FP, tag="ps_a")
            c0_ps = ps0.tile([BSH, H], FP, tag="ps_b")
            for kt in range(4):
                nc.tensor.matmul(
                    h0_ps, lhsT=fbT[:, kt, :], rhs=sb_winh[:, kt, :],
                    start=(kt == 0), stop=(kt == 3),
                )
            for kt in range(4):
                nc.tensor.matmul(
                    c0_ps, lhsT=fbT[:, kt, :], rhs=sb_winc[:, kt, :],
                    start=(kt == 0), stop=(kt == 3),
                )
            nc.scalar.copy(out=c_sb, in_=c0_ps)
            h0_sb = w0.tile([BSH, H], BF)
            nc.scalar.copy(out=h0_sb, in_=h0_ps)
            tp0 = tps0.tile([128, 4 * BSH], BF, name="tp")
            for kt in range(4):
                nc.tensor.transpose(
                    tp0[:, kt * BSH : (kt + 1) * BSH],
                    h0_sb[:, kt * 128 : (kt + 1) * 128],
                    sb_i16,
                )
            nc.scalar.copy(
                out=hallT[:, :, 0:BSH], in_=tp0.rearrange("p (k b) -> p k b", k=4)
            )

            # E = exp(att_v) * padmask   (max-sub skipped: |att_v| < ~3)
            e_sb = w0.tile([128, NLO], FP)
            nc.scalar.activation(out=e_sb, in_=attv, func=AF.Exp)
            nc.vector.tensor_mul(out=e_sb, in0=e_sb, in1=sb_padmask)
            esum = w0.tile([128, 1], FP)
            nc.vector.tensor_reduce(out=esum, in_=e_sb, axis=AX.X, op=OP.add)
            esum_bf = w0.tile([128, 1], BF)
            nc.vector.tensor_copy(out=esum_bf, in_=esum)
            den_ps = ps0.tile([BSH, 1], FP, tag="ps_a")
            nc.tensor.matmul(den_ps, lhsT=sb_onesbd, rhs=esum_bf, start=True, stop=True)
            rden = w0.tile([BSH, 1], FP)
            nc.vector.reciprocal(out=rden, in_=den_ps)

            # ctx (unnormalized): G = F*E (bf16), block-diag-ones matmul
            ctx_ps = ps0.tile([BSH, VD], FP, tag="ps_a")
            for nlo in range(NLO):
                g = g0.tile([128, VD], BF, name="g")
                nc.vector.tensor_scalar_mul(
                    out=g, in0=f_sb[:, nlo, :], scalar1=e_sb[:, nlo : nlo + 1]
                )
                nc.tensor.matmul(
                    ctx_ps, lhsT=sb_onesbd, rhs=g,
                    start=(nlo == 0), stop=(nlo == NLO - 1),
                )
            ctx_sb = w0.tile([BSH, VD], BF)
            nc.vector.tensor_scalar_mul(out=ctx_sb, in0=ctx_ps, scalar1=rden)
            ctxT = w0.tile([128, 4, BSH], BF)
            tpc = tps0.tile([128, 4 * BSH], BF, name="tp")
            for kt in range(4):
                nc.tensor.transpose(
                    tpc[:, kt * BSH : (kt + 1) * BSH],
                    ctx_sb[:, kt * 128 : (kt + 1) * 128],
                    sb_i16,
                )
            nc.scalar.copy(out=ctxT, in_=tpc.rearrange("p (k b) -> p k b", k=4))

            # ctx columns replicated across t: [128, 4, T*BSH]
            ctxRepT = w0.tile([128, 4, T * BSH], BF)
            ctx_b = bass.AP(
                tensor=ctxT.tensor,
                offset=ctxT.offset,
                ap=[ctxT.ap[0], ctxT.ap[1], [0, T], ctxT.ap[2]],
            )
            nc.vector.tensor_copy(
                out=ctxRepT.rearrange("p k (t b) -> p k t b", t=T), in_=ctx_b
            )
            embt_flat = sb_embt.rearrange("p a t b -> p (a t b)")

            # GE[t*16+b, :] = ctx@W_ihc^T + emb_t@W_ihe^T + (b_ih + b_hh)
            for m0, ml in CHUNKS:
                ge_ps = ps0.tile([128, G4], FP, tag="ps_big")
                for ns in range(4):
                    nsl = slice(ns * 512, (ns + 1) * 512)
                    for et in range(2):
                        e0 = et * T * BSH + m0
                        nc.tensor.matmul(
                            ge_ps[0:ml, nsl],
                            lhsT=embt_flat[:, e0 : e0 + ml],
                            rhs=sb_wihe[:, et, nsl],
                            start=(et == 0), stop=False,
                        )
                    for kt in range(4):
                        nc.tensor.matmul(
                            ge_ps[0:ml, nsl],
                            lhsT=ctxRepT[:, kt, m0 : m0 + ml],
                            rhs=sb_wihc[:, kt, nsl],
                            start=False, stop=False,
                        )
                    nc.tensor.matmul(
                        ge_ps[0:ml, nsl],
                        lhsT=sb_onesrow[0:1, 0:ml],
                        rhs=sb_biasrow[0:1, nsl],
                        start=False, stop=True,
                    )
                ge_sb = g0.tile([128, G4], BF, name="ge_sb", bufs=2)
                nc.scalar.copy(out=ge_sb[0:ml, :], in_=ge_ps[0:ml, :])
                nc.sync.dma_start(out=d_ge[m0 : m0 + ml, :], in_=ge_sb[0:ml, :])

        # ------- phases 1+2 interleaved: LSTM + vocab projection --------
        with ExitStack() as p12:
            whp = p12.enter_context(tc.tile_pool(name="whp", bufs=1))
            sb_whh = whp.tile([128, 4, G4], BF)
            nc.sync.dma_start(out=sb_whh, in_=d_whh)
            gein = p12.enter_context(tc.tile_pool(name="gein", bufs=2))
            gps = p12.enter_context(tc.tile_pool(name="gps", bufs=1, space="PSUM"))
            tps1 = p12.enter_context(tc.tile_pool(name="tps1", bufs=2, space="PSUM"))
            apool = p12.enter_context(tc.tile_pool(name="apool", bufs=1))
            ep = p12.enter_context(tc.tile_pool(name="ep", bufs=1))
            wop = p12.enter_context(tc.tile_pool(name="wop", bufs=4))
            bop = p12.enter_context(tc.tile_pool(name="bop", bufs=4))
            ps2 = p12.enter_context(tc.tile_pool(name="ps2", bufs=2, space="PSUM"))
            sp = p12.enter_context(tc.tile_pool(name="sp", bufs=1))

            ebs, scols = [], []
            for ci in range(3):
                eb = ep.tile([128, VOC], FP, name=f"eb{ci}")
                sc = sp.tile([128, NVT], FP, name=f"sc{ci}")
                ebs.append(eb)
                scols.append(sc)

            def lstm_step(t):
                ge_t = gein.tile([BSH, G4], BF, name="ge_t")
                nc.sync.dma_start(out=ge_t, in_=d_ge[t * BSH : (t + 1) * BSH, :])
                gates = gps.tile([BSH, G4], FP, name="gates")
                hsl = slice(t * BSH, (t + 1) * BSH)
                acts = {}
                ig = apool.tile([BSH, H], FP, name="ig")
                # gate order after host permutation: (g, i, f, o)
                for ns in range(4):
                    nsl = slice(ns * 512, (ns + 1) * 512)
                    for kt in range(4):
                        nc.tensor.matmul(
                            gates[:, nsl],
                            lhsT=hallT[:, kt, hsl],
                            rhs=sb_whh[:, kt, nsl],
                            start=(kt == 0), stop=False,
                        )
                    nc.tensor.matmul(
                        gates[:, nsl], lhsT=sb_i16, rhs=ge_t[:, nsl],
                        start=False, stop=True,
                    )
                    # pointwise for this gate slice, pipelined under the
                    # next slice's matmuls
                    gt = apool.tile([BSH, H], FP, name=f"act{ns}")
                    fn = AF.Tanh if ns == 0 else AF.Sigmoid
                    nc.scalar.activation(out=gt, in_=gates[:, nsl], func=fn)
                    acts[ns] = gt
                    if ns == 1:
                        nc.vector.tensor_mul(out=ig, in0=acts[1], in1=acts[0])
                    elif ns == 2:
                        nc.vector.tensor_mul(out=c_sb, in0=acts[2], in1=c_sb)
                nc.vector.tensor_add(out=c_sb, in0=c_sb, in1=ig)
                th = apool.tile([BSH, H], FP, name="th")
                nc.scalar.activation(out=th, in_=c_sb, func=AF.Tanh)
                nc.vector.tensor_mul(out=h_sb, in0=acts[3], in1=th)
                tp1 = tps1.tile([128, 4 * BSH], BF, name="tp1")
                for kt in range(4):
                    nc.tensor.transpose(
                        tp1[:, kt * BSH : (kt + 1) * BSH],
                        h_sb[:, kt * 128 : (kt + 1) * 128],
                        sb_i16,
                    )
                nc.scalar.copy(
                    out=hallT[:, :, (t + 1) * BSH : (t + 2) * BSH],
                    in_=tp1.rearrange("p (k b) -> p k b", k=4),
                )

            def p2block(ci, vts):
                m0, ml = CHUNKS[ci]
                for vt in vts:
                    vsl = slice(vt * VT, (vt + 1) * VT)
                    wo_t = wop.tile([128, 4, VT], BF, name="wo_t")
                    nc.sync.dma_start(out=wo_t, in_=d_wot[:, :, vsl])
                    bo_t = bop.tile([1, VT], BF, name="bo_t")
                    nc.sync.dma_start(out=bo_t, in_=d_borow[0:1, vsl])
                    ps = ps2.tile([128, VT], FP, name="ps")
                    for kt in range(4):
                        nc.tensor.matmul(
                            ps[0:ml, :],
                            lhsT=hallT[:, kt, BSH + m0 : BSH + m0 + ml],
                            rhs=wo_t[:, kt, :],
                            start=(kt == 0), stop=False,
                        )
                    nc.tensor.matmul(
                        ps[0:ml, :], lhsT=sb_onesrow[0:1, 0:ml], rhs=bo_t,
                        start=False, stop=True,
                    )
                    nc.scalar.activation(
                        out=ebs[ci][0:ml, vsl],
                        in_=ps[0:ml, :],
                        func=AF.Exp,
                        accum_out=scols[ci][0:ml, vt : vt + 1],
                    )

            def p2fin(ci):
                m0, ml = CHUNKS[ci]
                s_t = sp.tile([128, 1], FP, name=f"s{ci}")
                nc.vector.tensor_reduce(
                    out=s_t[0:ml], in_=scols[ci][0:ml, :], axis=AX.X, op=OP.add
                )
                r_t = sp.tile([128, 1], FP, name=f"r{ci}")
                nc.vector.reciprocal(out=r_t[0:ml], in_=s_t[0:ml])
                nc.vector.tensor_scalar_mul(
                    out=ebs[ci][0:ml, :], in0=ebs[ci][0:ml, :], scalar1=r_t[0:ml]
                )
                for hf in range(2):
                    fsl = slice(hf * 5000, (hf + 1) * 5000)
                    nc.sync.dma_start(
                        out=d_sm[m0 : m0 + ml, fsl], in_=ebs[ci][0:ml, fsl]
                    )
                nc.scalar.activation(
                    out=ebs[ci][0:ml, :], in_=ebs[ci][0:ml, :], func=AF.Ln
                )
                for hf in range(2):
                    fsl = slice(hf * 5000, (hf + 1) * 5000)
                    nc.sync.dma_start(
                        out=d_lsm[m0 : m0 + ml, fsl], in_=ebs[ci][0:ml, fsl]
                    )

            # steps 0..7: chunk-0 h states accumulate
            for t in range(8):
                lstm_step(t)
            # steps 8..15: interleave chunk-0 vocab tiles (2-3 per step)
            vt_sched0 = [2, 2, 2, 2, 3, 3, 3, 3]
            v = 0
            for i, t in enumerate(range(8, 16)):
                lstm_step(t)
                p2block(0, range(v, v + vt_sched0[i]))
                v += vt_sched0[i]
            p2fin(0)
            # steps 16..19: interleave chunk-1 vocab tiles (5 per step)
            v = 0
            for t in range(16, 20):
                lstm_step(t)
                p2block(1, range(v, v + 5))
                v += 5
            p2fin(1)
            p2block(2, range(NVT))
            p2fin(2)

    nc.compile()
    return nc


def _prep_host(inputs):
    import ml_dtypes

    f32 = np.float32
    bf16 = ml_dtypes.bfloat16
    feats = np.asarray(inputs["features"], f32)  # [128,196,512]
    caps = np.asarray(inputs["captions"]).astype(np.int64)
    emb_table = np.asarray(inputs["embed_table"], f32)
    emb = emb_table[caps]  # [128,20,256]

    W_ih = np.asarray(inputs["W_ih"], f32)  # [2048, 768]
    W_hh = np.asarray(inputs["W_hh"], f32)  # [2048, 512]
    Wo = np.asarray(inputs["Wo"], f32)  # [10000, 512]

    # permute gate rows: torch (i, f, g, o) -> (g, i, f, o)
    perm = np.concatenate(
        [np.arange(1024, 1536), np.arange(0, 512), np.arange(512, 1024),
         np.arange(1536, 2048)]
    )
    W_ih = W_ih[perm]
    W_hh = W_hh[perm]
    bias = (np.asarray(inputs["b_ih"], f32) + np.asarray(inputs["b_hh"], f32))[perm]

    def kxm(w_t, ktiles, ncols, dt=bf16):
        # w_t: [K, N] (already transposed weight) -> [128, ktiles, N]
        return np.ascontiguousarray(
            w_t.reshape(ktiles, 128, ncols).transpose(1, 0, 2).astype(dt)
        )

    shared = {
        "whh": kxm(W_hh.T.copy(), 4, G4),
        "wihe": kxm(np.ascontiguousarray(W_ih[:, VD:].T), 2, G4),
        "wihc": kxm(np.ascontiguousarray(W_ih[:, :VD].T), 4, G4),
        "winh": kxm(np.asarray(inputs["W_init_h"], f32).T.copy(), 4, H),
        "winc": kxm(np.asarray(inputs["W_init_c"], f32).T.copy(), 4, H),
        "wot": kxm(Wo.T.copy(), 4, VOC),
        "biasrow": np.ascontiguousarray(bias.reshape(1, G4).astype(bf16)),
        "borow": np.ascontiguousarray(
            np.asarray(inputs["bo"], f32).reshape(1, VOC).astype(bf16)
        ),
        "wvb": np.ascontiguousarray(
            np.broadcast_to(np.asarray(inputs["Wv"], f32).reshape(1, VD), (128, VD))
        ),
        "onesbd": np.ascontiguousarray(
            (np.arange(128)[:, None] // NHI == np.arange(BSH)[None, :]).astype(bf16)
        ),
        "i16": np.eye(BSH, dtype=bf16),
        "onesrow": np.ones((1, 128), bf16),
        "padmask": np.ascontiguousarray(
            (
                (np.arange(128)[:, None] % NHI) * NLO + np.arange(NLO)[None, :] < NVIS
            ).astype(f32)
        ),
    }

    in_maps = []
    for c in range(NCORES):
        fc = feats[c * BSH : (c + 1) * BSH]  # [16,196,512]
        fpad = np.zeros((BSH, NHI * NLO, VD), f32)
        fpad[:, :NVIS] = fc
        f_host = np.ascontiguousarray(fpad.reshape(128, NLO, VD))
        emb_c = emb[c * BSH : (c + 1) * BSH]  # [16,20,256]
        embt = np.ascontiguousarray(
            emb_c.transpose(2, 1, 0)
            .reshape(2, 128, T, BSH)
            .transpose(1, 0, 2, 3)
            .astype(bf16)
        )
        in_maps.append({"f": f_host, "embt": embt, **shared})
    return in_maps


def run_with_results(inputs, trace=False):
    from concourse.bass_utils import run_bass_kernel_spmd

    nc = _build_nc()
    in_maps = _prep_host(inputs)
    res = run_bass_kernel_spmd(
        nc, in_maps, core_ids=list(range(NCORES)), trace=trace
    )
    lsm_cores = np.stack([r["out_lsm"] for r in res.results])  # [8, 320, 10000]
    sm_cores = np.stack([r["out_sm"] for r in res.results])

    def assemble(a):
        # [8 cores, 20*16, V] -> time-major [T*B, V] with row = t*128 + b_global
        return np.ascontiguousarray(
            a.reshape(NCORES, T, BSH, VOC).transpose(1, 0, 2, 3).reshape(T * B, VOC)
        )

    return (assemble(lsm_cores), assemble(sm_cores)), res


def kernel(**inputs):
    outs, _ = run_with_results(inputs, trace=False)
    return outs
